# revision 50
# baseline (speedup 1.0000x reference)
"""Bass/Trainium2 kernel for CausalSelfAttention (B=8, T=1024, C=768, H=12).

Sharding: data-parallel over batch. 8 cores, one batch element per core.
No collectives. Each core runs an identical SPMD program on its own slice.

Per-core layouts (host-prepared):
  xT   [768, 1024] bf16   x[b].T
  wqk  [768, 1536] bf16   W_attn[:, :1536], Q columns pre-scaled by 1/sqrt(64)
  wv   [768, 768]  bf16   W_attn[:, 1536:]
  wp   [12, 64, 768] bf16 W_proj reshaped into 64-row tiles
  bqk  [128, 12]  f32     b_attn[:1536] per-tile columns (Q part pre-scaled)
  bv   [128, 768] f32     b_attn[1536:] broadcast over partitions
  bp   [128, 768] f32     b_proj broadcast over partitions
  qm   [128, 8]   f32     query_mask as per-partition columns per q-tile
  dm   [128, 8, 128] bf16 diagonal-block multiplicative masks, transposed
Output: y [1024, 768] f32 per core.
"""

import sys

if "/opt/trn_rl_repo" not in sys.path:
    sys.path.insert(0, "/opt/trn_rl_repo")

import numpy as np
import ml_dtypes

import concourse.bass as bass
import concourse.bacc as bacc
import concourse.mybir as mybir
import concourse.tile as tile
from concourse.bass import ts, ds

BF16 = mybir.dt.bfloat16
F32 = mybir.dt.float32
AF = mybir.ActivationFunctionType
ALU = mybir.AluOpType
BF16NP = ml_dtypes.bfloat16

T, C, H, HD = 1024, 768, 12, 64
NCORES = 8

_CACHE = {}


def build_program():
    """Build the single-core SPMD Bass program."""
    nc = bacc.Bacc("TRN2", target_bir_lowering=False, debug=False)

    xT_d = nc.dram_tensor("xT", [C, T], BF16, kind="ExternalInput")
    wqk_d = nc.dram_tensor("wqk", [C, 2 * C], BF16, kind="ExternalInput")
    wv_d = nc.dram_tensor("wv", [C, C], BF16, kind="ExternalInput")
    wp_d = nc.dram_tensor("wp", [C, C], BF16, kind="ExternalInput")
    bqk_d = nc.dram_tensor("bqk", [128, 12], F32, kind="ExternalInput")
    bv_d = nc.dram_tensor("bv", [128, C], F32, kind="ExternalInput")
    bp_d = nc.dram_tensor("bp", [128, C], F32, kind="ExternalInput")
    qm_d = nc.dram_tensor("qm", [128, 8], F32, kind="ExternalInput")
    dm_d = nc.dram_tensor("dm", [128, 8, 128], BF16, kind="ExternalInput")
    y_d = nc.dram_tensor("y", [T, C], F32, kind="ExternalOutput")

    with tile.TileContext(nc) as tc:
        with (
            tc.tile_pool(name="const", bufs=1) as cp,
            tc.tile_pool(name="ptp", bufs=10) as ptp,
            tc.tile_pool(name="recp", bufs=3) as recp,
            tc.tile_pool(name="bcp", bufs=3) as bcp,
            tc.tile_pool(name="otxp", bufs=3) as otxp,
            tc.tile_pool(name="ysb", bufs=3) as ysbp,
            tc.tile_pool(name="ps_a", bufs=5, space="PSUM") as ps_a,
            tc.tile_pool(name="ps_o", bufs=2, space="PSUM") as ps_o,
            tc.tile_pool(name="ps_bc", bufs=1, space="PSUM") as ps_bc,
        ):
            # ---------------- persistent SBUF tensors ----------------
            xT_sb = cp.tile([128, 6, T], BF16, name="xT_sb")
            wqk_sb = cp.tile([128, 6, 2 * C], BF16, name="wqk_sb")
            wv_sb = cp.tile([128, 6, C], BF16, name="wv_sb")
            wp_sb = cp.tile([128, 6, C], BF16, name="wp_sb")
            bqk_sb = cp.tile([128, 12], F32, name="bqk_sb")
            bv_sb = cp.tile([128, C], F32, name="bv_sb")
            bp_sb = cp.tile([128, C], F32, name="bp_sb")
            qm_sb = cp.tile([128, 8], F32, name="qm_sb")
            dm_sb = cp.tile([128, 8, 128], BF16, name="dm_sb")
            ones_sb = cp.tile([128, 64], F32, name="ones_sb")
            onesr_sb = cp.tile([128, 64], mybir.dt.float32r, name="onesr_sb")
            qk_sb = [cp.tile([128, T], BF16, name=f"qk{m}") for m in range(12)]
            v_sb = [cp.tile([128, 12 * 65], BF16, name=f"v{t}") for t in range(8)]
            ot_sb = cp.tile([128, 6, T], BF16, name="ot_sb")

            # ---------------- loads (split for DMA queue parallelism) ----------------
            nc.sync.dma_start(bqk_sb[:], bqk_d[:, :])
            nc.sync.dma_start(bv_sb[:], bv_d[:, :])
            nc.sync.dma_start(bp_sb[:], bp_d[:, :])
            nc.sync.dma_start(qm_sb[:], qm_d[:, :])
            nc.sync.dma_start(dm_sb[:], dm_d[:, :, :])
            nc.gpsimd.memset(ones_sb[:], 1.0)
            nc.vector.tensor_copy(onesr_sb[:], ones_sb[:])
            # ones columns interleaved into V (produce softmax sums during PV)
            for t in range(8):
                nc.gpsimd.memset(
                    v_sb[t].rearrange("p (h d) -> p h d", d=65)[:, :, 64:65], 1.0
                )
            for k in range(6):
                for c in range(2):
                    nc.sync.dma_start(
                        xT_sb[:, k, ts(c, 512)], xT_d[ts(k, 128), ts(c, 512)]
                    )
            # column-major, ordered to unlock m=0 (Q head 0/1) and m=6 (K
            # head 0/1) first
            for c in (0, 2, 1, 3):
                for k in range(6):
                    nc.sync.dma_start(
                        wqk_sb[:, k, ts(c, 384)], wqk_d[ts(k, 128), ts(c, 384)]
                    )
            for k in range(6):
                for c in range(2):
                    nc.sync.dma_start(
                        wv_sb[:, k, ts(c, 384)], wv_d[ts(k, 128), ts(c, 384)]
                    )
            for k in range(6):
                for c in range(2):
                    nc.sync.dma_start(
                        wp_sb[:, k, ts(c, 384)], wp_d[ts(k, 128), ts(c, 384)]
                    )


            # ---------------- phase B helper: one qkT m-tile ----------------
            def emit_qk(m):
                for j in range(2):
                    ps = ps_a.tile([128, 512], F32, name="ps", tag="a")
                    for k in range(6):
                        nc.tensor.matmul(
                            ps[:],
                            wqk_sb[:, k, ts(m, 128)],
                            xT_sb[:, k, ts(j, 512)],
                            start=(k == 0),
                            stop=(k == 5),
                        )
                    nc.scalar.activation(
                        qk_sb[m][:, ts(j, 512)],
                        ps[:],
                        AF.Identity,
                        bias=bqk_sb[:, m : m + 1],
                        scale=1.0,
                    )

            # first head-pair's projections before phase C: attention (and
            # the Scalar engine) start as early as possible
            emit_qk(0)
            emit_qk(6)

            # ---------------- phase C: V = x @ W_v + bv ----------------
            for t in range(8):
                for c0, cw in ((0, 512), (512, 256)):
                    psv = ps_a.tile([128, 512], F32, name="psv", tag="a")
                    for k in range(6):
                        nc.tensor.matmul(
                            psv[:, :cw],
                            xT_sb[:, k, ts(t, 128)],
                            wv_sb[:, k, ds(c0, cw)],
                            start=(k == 0),
                            stop=(k == 5),
                        )
                    nh, h0 = cw // 64, c0 // 64
                    nc.vector.tensor_add(
                        v_sb[t].rearrange("p (h d) -> p h d", d=65)[
                            :, h0 : h0 + nh, 0:64
                        ],
                        psv[:, :cw].rearrange("p (h d) -> p h d", d=64),
                        bv_sb[:, ds(c0, cw)].rearrange("p (h d) -> p h d", d=64),
                    )

            # ---------------- phase B+D interleaved per head-pair ----------------
            # kt's processed in pairs sharing a 2-bank [128,1024] psum tile:
            # half j holds S^T for kt=2i+j over the same 512 queries.
            for pr in range(6):
                if pr < 5:
                    emit_qk(pr + 1)
                    emit_qk(7 + pr)
                hs = (2 * pr, 2 * pr + 1)
                for sbi in range(2):
                    q0 = sbi * 512
                    nkt = 4 + 4 * sbi
                    psO = {}
                    for h in hs:
                        psO[h] = ps_o.tile([65, 512], F32, name="op", tag="op")
                    pts = {}
                    for kt in range(nkt):
                        dc = max(0, kt * 128 - q0)
                        w = 512 - dc
                        s_psum = {}
                        for h in hs:
                            qp = (h % 2) * 64
                            sp = ps_a.tile([128, 512], F32, name="sp", tag="a")
                            nc.tensor.matmul(
                                sp[:, ds(dc, w)],
                                qk_sb[6 + h // 2][qp : qp + 64, ts(kt, 128)],
                                qk_sb[h // 2][qp : qp + 64, ds(q0 + dc, w)],
                                start=True,
                                stop=True,
                            )
                            s_psum[h] = sp
                        for h in hs:
                            ptt = ptp.tile([128, 512], BF16, name="ptt", tag="ptt")
                            nc.scalar.activation(
                                ptt[:, ds(dc, w)],
                                s_psum[h][:, ds(dc, w)],
                                AF.Exp,
                            )
                            if kt * 128 >= q0:
                                nc.vector.tensor_mul(
                                    ptt[:, ds(dc, 128)],
                                    ptt[:, ds(dc, 128)],
                                    dm_sb[:, kt, :],
                                )
                            pts[(h, kt)] = ptt
                        for h in hs:
                            nc.tensor.matmul(
                                psO[h][:, ds(dc, w)],
                                v_sb[kt][:, h * 65 : h * 65 + 65],
                                pts[(h, kt)][:, ds(dc, w)],
                                start=(kt == 0),
                                stop=(kt == nkt - 1),
                                skip_group_check=True,
                            )
                    # normalize: OT = psO[0:64] / sum  (sum = psO row 64)
                    # sums -> sbuf f32r -> PE broadcast [64,512] -> approx recip
                    for h in hs:
                        sums = recp.tile(
                            [65, 512], mybir.dt.float32r, name="sums", tag="sums"
                        )
                        nc.vector.tensor_copy(sums[64:65, :], psO[h][64:65, :])
                        bc = ps_bc.tile([64, 512], F32, name="bc", tag="bc")
                        nc.tensor.matmul(
                            bc[:],
                            onesr_sb[64:65, 0:64],
                            sums[64:65, :],
                            start=True,
                            stop=True,
                        )
                        bcs = bcp.tile([64, 512], F32, name="bcs", tag="bcs")
                        nc.vector.reciprocal_approx_fast(bcs[:], bc[:])
                        j = h // 2
                        if h % 2 == 0:
                            nc.vector.tensor_mul(
                                ot_sb[0:64, j, ds(q0, 512)],
                                psO[h][0:64, :],
                                bcs[:],
                            )
                        else:
                            otx = otxp.tile([64, 512], BF16, name="otx", tag="otx")
                            nc.vector.tensor_mul(otx[:], psO[h][0:64, :], bcs[:])
                            nc.sync.dma_start(ot_sb[64:128, j, ds(q0, 512)], otx[:])

            # ---------------- phase E: y = OT.T @ W_proj * qm + bp ----------------
            for qt in range(8):
                ysb = ysbp.tile([128, C], F32, name="ysb", tag="ysb")
                for c0, cw in ((0, 512), (512, 256)):
                    psy = ps_a.tile([128, 512], F32, name="psy", tag="a")
                    for k in range(6):
                        nc.tensor.matmul(
                            psy[:, :cw],
                            ot_sb[:, k, ts(qt, 128)],
                            wp_sb[:, k, ds(c0, cw)],
                            start=(k == 0),
                            stop=(k == 5),
                        )
                    nc.vector.scalar_tensor_tensor(
                        out=ysb[:, ds(c0, cw)],
                        in0=psy[:, :cw],
                        scalar=qm_sb[:, qt : qt + 1],
                        in1=bp_sb[:, ds(c0, cw)],
                        op0=ALU.mult,
                        op1=ALU.add,
                    )
                nc.sync.dma_start(y_d[ts(qt, 128), :], ysb[:])

    nc.compile()
    return nc


def _get_nc():
    if "nc" not in _CACHE:
        _CACHE["nc"] = build_program()
    return _CACHE["nc"]


def prep_core_inputs(x, mask, query_mask, W_attn, b_attn, W_proj, b_proj):
    """Host-side prep. Returns (shared, per_core) where per_core is a list of
    dicts for each batch element."""
    scale = 1.0 / np.sqrt(HD)
    W_s = np.asarray(W_attn, np.float32).copy()
    W_s[:, :C] *= scale
    b_s = np.asarray(b_attn, np.float32).copy()
    b_s[:C] *= scale

    shared = {
        "wqk": W_s[:, : 2 * C].astype(BF16NP),
        "wv": W_s[:, 2 * C :].astype(BF16NP),
        "wp": np.asarray(W_proj, np.float32).astype(BF16NP),
        "bqk": np.ascontiguousarray(b_s[: 2 * C].reshape(12, 128).T),
        "bv": np.ascontiguousarray(
            np.broadcast_to(b_s[2 * C :], (128, C))
        ).astype(np.float32),
        "bp": np.ascontiguousarray(
            np.broadcast_to(np.asarray(b_proj, np.float32), (128, C))
        ),
    }

    per_core = []
    for b in range(NCORES):
        xT = np.ascontiguousarray(np.asarray(x[b], np.float32).T).astype(BF16NP)
        qm = np.ascontiguousarray(
            np.asarray(query_mask[b, 0, :, 0], np.float32).reshape(8, 128).T
        )
        mb = np.asarray(mask[b, 0])  # [T, T] bool
        blocks = [
            mb[qi * 128 : (qi + 1) * 128, qi * 128 : (qi + 1) * 128].T
            for qi in range(8)
        ]
        dm = np.stack(blocks, axis=1).astype(BF16NP)  # [128, 8, 128]
        per_core.append({"xT": xT, "qm": qm, "dm": dm, **shared})
    return per_core


def run_on_cores(inputs, trace=False, **kw):
    from concourse.bass_utils import run_bass_kernel_spmd

    nc = _get_nc()
    in_maps = prep_core_inputs(**inputs)
    res = run_bass_kernel_spmd(
        nc, in_maps, core_ids=list(range(NCORES)), trace=trace, **kw
    )
    out = np.stack([res.results[b]["y"] for b in range(NCORES)], axis=0)
    return out.astype(np.float32), res


def kernel(**inputs) -> np.ndarray:
    out, _ = run_on_cores(inputs, trace=False)
    return out


# revision 51
# speedup vs baseline: 1.0092x; 1.0092x over previous
"""Bass/Trainium2 kernel for CausalSelfAttention (B=8, T=1024, C=768, H=12).

Sharding: data-parallel over batch. 8 cores, one batch element per core.
No collectives. Each core runs an identical SPMD program on its own slice.

Per-core layouts (host-prepared):
  xT   [768, 1024] bf16   x[b].T
  wqk  [768, 1536] bf16   W_attn[:, :1536], Q columns pre-scaled by 1/sqrt(64)
  wv   [768, 768]  bf16   W_attn[:, 1536:]
  wp   [12, 64, 768] bf16 W_proj reshaped into 64-row tiles
  bqk  [128, 12]  f32     b_attn[:1536] per-tile columns (Q part pre-scaled)
  bv   [128, 768] f32     b_attn[1536:] broadcast over partitions
  bp   [128, 768] f32     b_proj broadcast over partitions
  qm   [128, 8]   f32     query_mask as per-partition columns per q-tile
  dm   [128, 8, 128] bf16 diagonal-block multiplicative masks, transposed
Output: y [1024, 768] f32 per core.
"""

import sys

if "/opt/trn_rl_repo" not in sys.path:
    sys.path.insert(0, "/opt/trn_rl_repo")

import numpy as np
import ml_dtypes

import concourse.bass as bass
import concourse.bacc as bacc
import concourse.mybir as mybir
import concourse.tile as tile
from concourse.bass import ts, ds

BF16 = mybir.dt.bfloat16
F32 = mybir.dt.float32
AF = mybir.ActivationFunctionType
ALU = mybir.AluOpType
BF16NP = ml_dtypes.bfloat16

T, C, H, HD = 1024, 768, 12, 64
NCORES = 8

_CACHE = {}


def build_program():
    """Build the single-core SPMD Bass program."""
    nc = bacc.Bacc("TRN2", target_bir_lowering=False, debug=False)

    xT_d = nc.dram_tensor("xT", [C, T], BF16, kind="ExternalInput")
    wqk_d = nc.dram_tensor("wqk", [C, 2 * C], BF16, kind="ExternalInput")
    wv_d = nc.dram_tensor("wv", [C, C], BF16, kind="ExternalInput")
    wp_d = nc.dram_tensor("wp", [C, C], BF16, kind="ExternalInput")
    bqk_d = nc.dram_tensor("bqk", [128, 12], F32, kind="ExternalInput")
    bv_d = nc.dram_tensor("bv", [128, C], F32, kind="ExternalInput")
    bp_d = nc.dram_tensor("bp", [128, C], F32, kind="ExternalInput")
    qm_d = nc.dram_tensor("qm", [128, 8], F32, kind="ExternalInput")
    dm_d = nc.dram_tensor("dm", [128, 8, 128], BF16, kind="ExternalInput")
    y_d = nc.dram_tensor("y", [T, C], F32, kind="ExternalOutput")

    with tile.TileContext(nc) as tc:
        with (
            tc.tile_pool(name="const", bufs=1) as cp,
            tc.tile_pool(name="ptp", bufs=10) as ptp,
            tc.tile_pool(name="recp", bufs=3) as recp,
            tc.tile_pool(name="bcp", bufs=3) as bcp,
            tc.tile_pool(name="otxp", bufs=3) as otxp,
            tc.tile_pool(name="ysb", bufs=3) as ysbp,
            tc.tile_pool(name="ps_a", bufs=5, space="PSUM") as ps_a,
            tc.tile_pool(name="ps_o", bufs=2, space="PSUM") as ps_o,
            tc.tile_pool(name="ps_bc", bufs=1, space="PSUM") as ps_bc,
        ):
            # ---------------- persistent SBUF tensors ----------------
            xT_sb = cp.tile([128, 6, T], BF16, name="xT_sb")
            wqk_sb = cp.tile([128, 6, 2 * C], BF16, name="wqk_sb")
            wv_sb = cp.tile([128, 6, C], BF16, name="wv_sb")
            wp_sb = cp.tile([128, 6, C], BF16, name="wp_sb")
            bqk_sb = cp.tile([128, 12], F32, name="bqk_sb")
            bv_sb = cp.tile([128, C], F32, name="bv_sb")
            bp_sb = cp.tile([128, C], F32, name="bp_sb")
            qm_sb = cp.tile([128, 8], F32, name="qm_sb")
            dm_sb = cp.tile([128, 8, 128], BF16, name="dm_sb")
            ones_sb = cp.tile([128, 64], F32, name="ones_sb")
            onesr_sb = cp.tile([128, 64], mybir.dt.float32r, name="onesr_sb")
            qk_sb = [cp.tile([128, T], BF16, name=f"qk{m}") for m in range(12)]
            v_sb = [cp.tile([128, 12 * 65], BF16, name=f"v{t}") for t in range(8)]
            ot_sb = cp.tile([128, 6, T], BF16, name="ot_sb")

            # ---------------- loads ----------------
            # one dma_start per tensor (a single DMA's packets already fan
            # out across all HW DGE engines); issue from two engines so the
            # ~0.6us per-issue cost doesn't serialize the ramp
            nc.sync.dma_start(bqk_sb[:], bqk_d[:, :])
            nc.sync.dma_start(
                xT_sb[:, :, :], xT_d[:, :].rearrange("(k p) t -> p k t", p=128)
            )
            nc.sync.dma_start(
                wqk_sb[:, :, :], wqk_d[:, :].rearrange("(k p) m -> p k m", p=128)
            )
            nc.gpsimd.dma_start(qm_sb[:], qm_d[:, :])
            nc.gpsimd.dma_start(
                wv_sb[:, :, :], wv_d[:, :].rearrange("(k p) m -> p k m", p=128)
            )
            nc.gpsimd.dma_start(bv_sb[:], bv_d[:, :])
            nc.gpsimd.dma_start(dm_sb[:], dm_d[:, :, :])
            nc.gpsimd.dma_start(
                wp_sb[:, :, :], wp_d[:, :].rearrange("(k p) m -> p k m", p=128)
            )
            nc.gpsimd.dma_start(bp_sb[:], bp_d[:, :])
            nc.gpsimd.memset(ones_sb[:], 1.0)
            nc.vector.tensor_copy(onesr_sb[:], ones_sb[:])
            # ones columns interleaved into V (produce softmax sums during PV)
            for t in range(8):
                nc.gpsimd.memset(
                    v_sb[t].rearrange("p (h d) -> p h d", d=65)[:, :, 64:65], 1.0
                )


            # ---------------- phase B helper: one qkT m-tile ----------------
            def emit_qk(m):
                for j in range(2):
                    ps = ps_a.tile([128, 512], F32, name="ps", tag="a")
                    for k in range(6):
                        nc.tensor.matmul(
                            ps[:],
                            wqk_sb[:, k, ts(m, 128)],
                            xT_sb[:, k, ts(j, 512)],
                            start=(k == 0),
                            stop=(k == 5),
                        )
                    nc.scalar.activation(
                        qk_sb[m][:, ts(j, 512)],
                        ps[:],
                        AF.Identity,
                        bias=bqk_sb[:, m : m + 1],
                        scale=1.0,
                    )

            # first head-pair's projections before phase C: attention (and
            # the Scalar engine) start as early as possible
            emit_qk(0)
            emit_qk(6)

            # ---------------- phase C: V = x @ W_v + bv ----------------
            for t in range(8):
                for c0, cw in ((0, 512), (512, 256)):
                    psv = ps_a.tile([128, 512], F32, name="psv", tag="a")
                    for k in range(6):
                        nc.tensor.matmul(
                            psv[:, :cw],
                            xT_sb[:, k, ts(t, 128)],
                            wv_sb[:, k, ds(c0, cw)],
                            start=(k == 0),
                            stop=(k == 5),
                        )
                    nh, h0 = cw // 64, c0 // 64
                    nc.vector.tensor_add(
                        v_sb[t].rearrange("p (h d) -> p h d", d=65)[
                            :, h0 : h0 + nh, 0:64
                        ],
                        psv[:, :cw].rearrange("p (h d) -> p h d", d=64),
                        bv_sb[:, ds(c0, cw)].rearrange("p (h d) -> p h d", d=64),
                    )

            # ---------------- phase B+D interleaved per head-pair ----------------
            # kt's processed in pairs sharing a 2-bank [128,1024] psum tile:
            # half j holds S^T for kt=2i+j over the same 512 queries.
            for pr in range(6):
                if pr < 5:
                    emit_qk(pr + 1)
                    emit_qk(7 + pr)
                hs = (2 * pr, 2 * pr + 1)
                for sbi in range(2):
                    q0 = sbi * 512
                    nkt = 4 + 4 * sbi
                    psO = {}
                    for h in hs:
                        psO[h] = ps_o.tile([65, 512], F32, name="op", tag="op")
                    pts = {}
                    for kt in range(nkt):
                        dc = max(0, kt * 128 - q0)
                        w = 512 - dc
                        s_psum = {}
                        for h in hs:
                            qp = (h % 2) * 64
                            sp = ps_a.tile([128, 512], F32, name="sp", tag="a")
                            nc.tensor.matmul(
                                sp[:, ds(dc, w)],
                                qk_sb[6 + h // 2][qp : qp + 64, ts(kt, 128)],
                                qk_sb[h // 2][qp : qp + 64, ds(q0 + dc, w)],
                                start=True,
                                stop=True,
                            )
                            s_psum[h] = sp
                        for h in hs:
                            ptt = ptp.tile([128, 512], BF16, name="ptt", tag="ptt")
                            nc.scalar.activation(
                                ptt[:, ds(dc, w)],
                                s_psum[h][:, ds(dc, w)],
                                AF.Exp,
                            )
                            if kt * 128 >= q0:
                                nc.vector.tensor_mul(
                                    ptt[:, ds(dc, 128)],
                                    ptt[:, ds(dc, 128)],
                                    dm_sb[:, kt, :],
                                )
                            pts[(h, kt)] = ptt
                        for h in hs:
                            nc.tensor.matmul(
                                psO[h][:, ds(dc, w)],
                                v_sb[kt][:, h * 65 : h * 65 + 65],
                                pts[(h, kt)][:, ds(dc, w)],
                                start=(kt == 0),
                                stop=(kt == nkt - 1),
                                skip_group_check=True,
                            )
                    # normalize: OT = psO[0:64] / sum  (sum = psO row 64)
                    # sums -> sbuf f32r -> PE broadcast [64,512] -> approx recip
                    for h in hs:
                        sums = recp.tile(
                            [65, 512], mybir.dt.float32r, name="sums", tag="sums"
                        )
                        nc.vector.tensor_copy(sums[64:65, :], psO[h][64:65, :])
                        bc = ps_bc.tile([64, 512], F32, name="bc", tag="bc")
                        nc.tensor.matmul(
                            bc[:],
                            onesr_sb[64:65, 0:64],
                            sums[64:65, :],
                            start=True,
                            stop=True,
                        )
                        bcs = bcp.tile([64, 512], F32, name="bcs", tag="bcs")
                        nc.vector.reciprocal_approx_fast(bcs[:], bc[:])
                        j = h // 2
                        if h % 2 == 0:
                            nc.vector.tensor_mul(
                                ot_sb[0:64, j, ds(q0, 512)],
                                psO[h][0:64, :],
                                bcs[:],
                            )
                        else:
                            otx = otxp.tile([64, 512], BF16, name="otx", tag="otx")
                            nc.vector.tensor_mul(otx[:], psO[h][0:64, :], bcs[:])
                            nc.sync.dma_start(ot_sb[64:128, j, ds(q0, 512)], otx[:])

            # ---------------- phase E: y = OT.T @ W_proj * qm + bp ----------------
            for qt in range(8):
                ysb = ysbp.tile([128, C], F32, name="ysb", tag="ysb")
                for c0, cw in ((0, 512), (512, 256)):
                    psy = ps_a.tile([128, 512], F32, name="psy", tag="a")
                    for k in range(6):
                        nc.tensor.matmul(
                            psy[:, :cw],
                            ot_sb[:, k, ts(qt, 128)],
                            wp_sb[:, k, ds(c0, cw)],
                            start=(k == 0),
                            stop=(k == 5),
                        )
                    nc.vector.scalar_tensor_tensor(
                        out=ysb[:, ds(c0, cw)],
                        in0=psy[:, :cw],
                        scalar=qm_sb[:, qt : qt + 1],
                        in1=bp_sb[:, ds(c0, cw)],
                        op0=ALU.mult,
                        op1=ALU.add,
                    )
                nc.sync.dma_start(y_d[ts(qt, 128), :], ysb[:])

    nc.compile()
    return nc


def _get_nc():
    if "nc" not in _CACHE:
        _CACHE["nc"] = build_program()
    return _CACHE["nc"]


def prep_core_inputs(x, mask, query_mask, W_attn, b_attn, W_proj, b_proj):
    """Host-side prep. Returns (shared, per_core) where per_core is a list of
    dicts for each batch element."""
    scale = 1.0 / np.sqrt(HD)
    W_s = np.asarray(W_attn, np.float32).copy()
    W_s[:, :C] *= scale
    b_s = np.asarray(b_attn, np.float32).copy()
    b_s[:C] *= scale

    shared = {
        "wqk": W_s[:, : 2 * C].astype(BF16NP),
        "wv": W_s[:, 2 * C :].astype(BF16NP),
        "wp": np.asarray(W_proj, np.float32).astype(BF16NP),
        "bqk": np.ascontiguousarray(b_s[: 2 * C].reshape(12, 128).T),
        "bv": np.ascontiguousarray(
            np.broadcast_to(b_s[2 * C :], (128, C))
        ).astype(np.float32),
        "bp": np.ascontiguousarray(
            np.broadcast_to(np.asarray(b_proj, np.float32), (128, C))
        ),
    }

    per_core = []
    for b in range(NCORES):
        xT = np.ascontiguousarray(np.asarray(x[b], np.float32).T).astype(BF16NP)
        qm = np.ascontiguousarray(
            np.asarray(query_mask[b, 0, :, 0], np.float32).reshape(8, 128).T
        )
        mb = np.asarray(mask[b, 0])  # [T, T] bool
        blocks = [
            mb[qi * 128 : (qi + 1) * 128, qi * 128 : (qi + 1) * 128].T
            for qi in range(8)
        ]
        dm = np.stack(blocks, axis=1).astype(BF16NP)  # [128, 8, 128]
        per_core.append({"xT": xT, "qm": qm, "dm": dm, **shared})
    return per_core


def run_on_cores(inputs, trace=False, **kw):
    from concourse.bass_utils import run_bass_kernel_spmd

    nc = _get_nc()
    in_maps = prep_core_inputs(**inputs)
    res = run_bass_kernel_spmd(
        nc, in_maps, core_ids=list(range(NCORES)), trace=trace, **kw
    )
    out = np.stack([res.results[b]["y"] for b in range(NCORES)], axis=0)
    return out.astype(np.float32), res


def kernel(**inputs) -> np.ndarray:
    out, _ = run_on_cores(inputs, trace=False)
    return out
